# revision 18
# baseline (speedup 1.0000x reference)
"""Trainium2 Bass kernel for a single causal attention head.

Problem: x:[8,2048,1024] f32, Wq/Wk/Wv:[64,1024], causal mask.
  Q = x@Wq.T; K = x@Wk.T; V = x@Wv.T
  out = softmax(mask(Q@K.T/sqrt(64))) @ V          -> [8, 2048, 64] f32

Sharding: data-parallel over batch. B == n_cores == 8, one batch element
per NeuronCore; no collectives.

Per-core algorithm (fp16 matmul inputs, fp32 PSUM accumulation):
  - DMA plan: weights land first on the Act HWDGE queue, xT chunks are
    split across the SP and Act queues so projections chase the DMA.
  - Projections chunk-major over the contraction (e) so the PE consumes
    each xT chunk as it lands: QT/VT fused ([Wq;Wv] -> 128 psum rows),
    KT via column-tiled matmul pairs (two jb q-blocks concurrently in
    the 128x64-tiled PE array).
  - kt is zero-padded to 128 partitions so the scores matmuls run in
    full 128x128 mode (no PE mode switches): scoresT[k,q] over a k x q
    block grid with causal skipping; on diagonal blocks only the
    unmasked column window is computed/exp'd, and only the leading
    128 cols of each window need the staircase multiplicative mask
    (generated on-device via gpsimd affine_select).
  - exp on ScalarE straight out of PSUM, merged across two 512-col
    blocks per instruction ([128,1024] spans two PSUM banks) to
    amortize ACT overhead; a dummy exp at t=0 preloads the ACT table.
  - V tiles are PE-transposed VT->V into V_aug [128, 65] tiles
    (ones col 64), interleaved per q-block with the attention stream.
  - out_augT[65,q] = V_aug.T @ exp accumulated over k: rows 0:64 are
    unnormalized outT, row 64 the softmax denominator Z.
  - normalize: PE-transpose out_augT back to [128, 65] tiles (Z becomes
    a column), DVE per-partition reciprocal + scale, DMA out [S, 64]
    fp16 directly (no host transpose).
"""

import numpy as np

B, S, E, D = 8, 2048, 1024, 64
NCORES = 8
EC = E // 128   # 8 e-chunks
ST = S // 128   # 16 k-tiles
QB = S // 512   # 4 q-blocks

_cache = {}


def _patch_tile_drain():
    """The pinned walrus rejects >~2 sem waits on one Drain; Tile's tail
    drain waits on every live semaphore.  Split the excess onto standalone
    wait_ge instructions (same semantics: all waits complete before the
    all-engine barrier resets semaphores)."""
    import concourse.mybir as mybir
    import concourse.tile as ctile
    from concourse.vector_clock import ScopedClock

    if getattr(ctile.TileContext, "_drain_patch", False):
        return

    def _drain_and_barrier(self, tick_clock, wait_clock):
        nc = self.nc
        drain_inst = nc.sync.drain()
        wait_clock.add_sem_waits(
            drain_inst.ins, ScopedClock({None: tick_clock.global_clock})
        )
        si = drain_inst.ins.sync_info
        if si is not None and si.on_wait and len(si.on_wait) > 1:
            waits = list(si.on_wait)
            drain_inst.ins.sync_info = mybir.SyncInfo(
                on_wait=[waits[0]], on_update=list(si.on_update)
            )
            handles = {h.num: h for h in self.sems.allocated().values()}
            for w in waits[1:]:
                assert w.wait_mode == "sem-ge-imm", w
                nc.sync.wait_ge(handles[w.id], w.wait_value)
        nc.all_engine_barrier()
        popped = nc._tile_sem_poison_stack.pop()
        assert popped is self._sem_poison
        nc.clear_and_free_semaphores(list(self.sems.allocated().values()))
        nc.all_engine_barrier()

    ctile.TileContext._drain_and_barrier = _drain_and_barrier
    ctile.TileContext._drain_patch = True


def _split_sync_waits(nc, maxw=1):
    """The pinned walrus rejects instructions carrying more than ~2 sem
    waits.  Hoist all but `maxw` waits of every instruction onto dedicated
    NoOps just before it in the same engine stream (engine streams are
    in-order, so semantics are identical)."""
    import concourse.mybir as mybir

    n_new = 0
    for f in nc.m.functions:
        for b in f.blocks:
            out = []
            changed = False
            for inst in b.instructions:
                si = getattr(inst, "sync_info", None)
                if si is not None and si.on_wait and len(si.on_wait) > maxw:
                    waits = list(si.on_wait)
                    extras, keep = waits[:-maxw], waits[-maxw:]
                    for k, w in enumerate(extras):
                        nop = mybir.InstNoOp(
                            name=f"{inst.name}-hw{k}", ins=[], outs=[],
                            sync_info=mybir.SyncInfo(on_wait=[w], on_update=[]),
                        )
                        nop.engine = inst.engine
                        nc.register_instruction(nop)
                        out.append(nop)
                        n_new += 1
                    inst.sync_info = mybir.SyncInfo(
                        on_wait=keep, on_update=list(si.on_update)
                    )
                    changed = True
                out.append(inst)
            if changed:
                b.instructions = out
    return n_new


def _att_groups(jb):
    """Attention (ki) block groups for q-block jb.  Each group is a list of
    (ki, psum_col, n, q_off, stair_cols) where the scores matmul writes
    psS[:, psum_col:psum_col+n] from rhs qv[:, jb*512+q_off : +n], and
    stair_cols lists ex-tile col offsets needing the [128,128] stair mul."""
    groups = []
    # full (off-diagonal) ki, in pairs
    full = list(range(4 * jb))
    for i in range(0, len(full), 2):
        a, b = full[i], full[i + 1]
        groups.append([(a, 0, 512, 0, None), (b, 512, 512, 0, None)])
    # diagonal blocks r=0..3 (ki = 4*jb + r): valid q window [128r, 512)
    # D1: r0 (n=512 @ col 0) + r1 (n=384 @ col 512)
    groups.append([
        (4 * jb + 0, 0, 512, 0, 0),
        (4 * jb + 1, 512, 384, 128, 512),
    ])
    # D2: r2 (n=256 @ col 0) + r3 (n=128 @ col 256)
    groups.append([
        (4 * jb + 2, 0, 256, 256, 0),
        (4 * jb + 3, 256, 128, 384, 256),
    ])
    return groups


def _build_nc():
    import concourse.bass as bass
    import concourse.mybir as mybir
    from concourse import tile

    _patch_tile_drain()

    fp16 = mybir.dt.float16
    f32 = mybir.dt.float32
    EXP = mybir.ActivationFunctionType.Exp

    nc = bass.Bass("TRN2", target_bir_lowering=False)
    # host pre-layout: xT = x.T fp16; weights pre-shuffled to [128, EC*cols]
    xT_d = nc.dram_tensor("xT", [E, S], fp16, kind="ExternalInput")
    wqv_d = nc.dram_tensor("wqvh", [128, EC * 128], fp16, kind="ExternalInput")
    wk_d = nc.dram_tensor("wkh", [128, EC * D], fp16, kind="ExternalInput")
    out_d = nc.dram_tensor("out", [S, D], fp16, kind="ExternalOutput")

    with tile.TileContext(nc) as tc:
        with (
            tc.tile_pool(name="singles", bufs=1) as singles,
            tc.tile_pool(name="expool", bufs=4) as expool,
            tc.tile_pool(name="misc", bufs=3) as misc,
            tc.tile_pool(name="psS", bufs=3, space="PSUM") as psS,
            tc.tile_pool(name="psO", bufs=2, space="PSUM") as psO,
        ):
            # ---- DMAs first: weights (contiguous), xT chunks split SP/Act ----
            wqv = singles.tile([128, EC, 128], fp16)
            wk = singles.tile([128, EC, D], fp16)
            nc.sync.dma_start(
                wqv[:], wqv_d[:].rearrange("p (e c) -> p e c", e=EC))
            nc.sync.dma_start(
                wk[:], wk_d[:].rearrange("p (e c) -> p e c", e=EC))
            # xT streamed in column halves: q-blocks 0/1 of every chunk land
            # first, so their projection+attention overlap the later half
            xts = [singles.tile([128, S], fp16, name=f"xt{ec}")
                   for ec in range(EC)]
            for half in range(2):
                hs = slice(half * 1024, (half + 1) * 1024)
                for ec in range(EC):
                    eng = nc.scalar if ec % 2 == 0 else nc.sync
                    eng.dma_start(xts[ec][:, hs],
                                  xT_d[ec * 128:(ec + 1) * 128, hs])

            # ---- t=0 setup (overlaps the DMA lead-in) ----
            warm = singles.tile([1, 16], fp16)
            nc.gpsimd.memset(warm[:], 0.0)
            nc.scalar.activation(warm[:], warm[:], EXP)  # exp table load at t~0

            stair = singles.tile([128, 128], fp16)  # stair[p,c] = 1 if p <= c
            nc.gpsimd.memset(stair[:], 1.0)
            nc.gpsimd.affine_select(
                out=stair[:], in_=stair[:],
                compare_op=mybir.AluOpType.is_ge, fill=0.0, base=0,
                pattern=[[1, 128]], channel_multiplier=-1,
            )
            from concourse.masks import make_identity
            ident = singles.tile([128, 128], fp16)
            make_identity(nc, ident[:])

            # kt: KT on partitions 0:64, zeros on 64:128 (scores matmuls run
            # K=128 full-array; VT rows in rhs are nulled by the zero weights)
            kt = singles.tile([128, S], fp16)
            nc.gpsimd.memset(kt[64:128, :], 0.0)

            vaug = singles.tile([128, ST, 65], fp16)
            for si in range(ST):
                nc.vector.memset(vaug[:, si, 64:65], 1.0)

            # ---- projections in two passes (q-blocks 0/1, then 2/3),
            # chunk-major within each pass to chase the DMA stream; K is
            # plain M=64 into its own bank (no cross-partition shuffles) ----
            qv_sb = singles.tile([128, S], fp16)

            def issue_proj_pass(jbs, tag_sfx):
                qvt = psS.tile([128, 1024], f32, tag="s", name=f"qv{tag_sfx}")
                ka = psO.tile([128, 512], f32, tag="o", name=f"ka{tag_sfx}")
                kb = psO.tile([128, 512], f32, tag="o", name=f"kb{tag_sfx}")
                kts = (ka, kb)
                for ec in range(EC):
                    st, sp = ec == 0, ec == EC - 1
                    for half, jb in enumerate(jbs):
                        nc.tensor.matmul(
                            qvt[:, half * 512:(half + 1) * 512], wqv[:, ec, :],
                            xts[ec][:, jb * 512:(jb + 1) * 512],
                            start=st, stop=sp, skip_group_check=True)
                    for half, jb in enumerate(jbs):
                        nc.tensor.matmul(
                            kts[half][0:64, :], wk[:, ec, :],
                            xts[ec][:, jb * 512:(jb + 1) * 512],
                            start=st, stop=sp, skip_group_check=True)
                for half, jb in enumerate(jbs):
                    nc.vector.tensor_copy(
                        qv_sb[:, jb * 512:(jb + 1) * 512],
                        qvt[:, half * 512:(half + 1) * 512])
                    nc.vector.tensor_copy(
                        kt[0:64, jb * 512:(jb + 1) * 512], kts[half][0:64, :])

            # ---- attention: flat software-pipelined group list ----
            sched = []
            for jb in range(QB):
                for g in _att_groups(jb):
                    sched.append((jb, g))

            ps_os = {}
            ex_of = {}
            vt_done = set()

            def issue_vt(jb):
                vt = psS.tile([128, 4, 64], fp16, tag="s", name=f"vt{jb}")
                for sub in range(4):
                    si = jb * 4 + sub
                    nc.tensor.transpose(
                        vt[:, sub, :], qv_sb[64:128, si * 128:(si + 1) * 128],
                        ident[64:128, 64:128])
                nc.vector.tensor_copy(vaug[:, jb * 4:(jb + 1) * 4, 0:64], vt[:])

            passB = [False]

            def issue_S(idx):
                jb, g = sched[idx]
                if jb >= 2 and not passB[0]:
                    passB[0] = True
                    issue_proj_pass((2, 3), "23")
                if jb not in vt_done:
                    vt_done.add(jb)
                    issue_vt(jb)
                pst = psS.tile([128, 1024], f32, tag="s", name=f"s{idx}")
                for (ki, pcol, n, qoff, _sc) in g:
                    nc.tensor.matmul(
                        pst[:, pcol:pcol + n],
                        kt[:, ki * 128:(ki + 1) * 128],
                        qv_sb[:, jb * 512 + qoff:jb * 512 + qoff + n],
                        start=True, stop=True, skip_group_check=True)
                span = max(pcol + n for (_k, pcol, n, _q, _s) in g)
                ex = expool.tile([128, 1024], fp16, tag="ex", name=f"ex{idx}")
                ex_of[idx] = ex
                nc.scalar.activation(ex[:, 0:span], pst[:, 0:span], EXP,
                                     scale=0.125)
                for (_ki, _pcol, _n, _qoff, sc) in g:
                    if sc is not None:
                        nc.gpsimd.tensor_mul(
                            ex[:, sc:sc + 128], ex[:, sc:sc + 128], stair[:])

            def issue_A(idx):
                jb, g = sched[idx]
                if jb not in ps_os:
                    ps_os[jb] = psO.tile([65, 512], f32, tag="o", name=f"o{jb}")
                po = ps_os[jb]
                ex = ex_of[idx]
                nki = 4 * jb + 4
                for (ki, pcol, n, qoff, _sc) in g:
                    nc.tensor.matmul(
                        po[:, qoff:qoff + n], vaug[:, ki, :],
                        ex[:, pcol:pcol + n],
                        start=(ki == 0), stop=(ki == nki - 1),
                        skip_group_check=True)

            def issue_norm(jb):
                # transpose out_augT back (Z becomes a column), DVE reciprocal
                # + scale per partition, one merged store per q-block
                po = ps_os[jb]
                posb = misc.tile([65, 512], fp16, tag="posb", name=f"posb{jb}")
                nc.vector.tensor_copy(posb[:], po[:])
                nt = psS.tile([128, 4, 66], fp16, tag="s", name=f"nt{jb}")
                for sub in range(4):
                    nc.tensor.transpose(
                        nt[:, sub, 0:65], posb[:, sub * 128:(sub + 1) * 128],
                        ident[0:65, 0:65])
                osb = misc.tile([128, 4, 64], fp16, tag="osb", name=f"osb{jb}")
                for sub in range(4):
                    rc = misc.tile([128, 1], f32, tag="rc", name=f"rc{jb}_{sub}")
                    nc.vector.reciprocal(rc[:], nt[:, sub, 64:65])
                    nc.vector.tensor_scalar_mul(osb[:, sub, :],
                                                nt[:, sub, 0:64], rc[:])
                nc.sync.dma_start(
                    out_d[jb * 512:(jb + 1) * 512, :].rearrange(
                        "(s p) d -> p s d", p=128),
                    osb[:])

            issue_proj_pass((0, 1), "01")

            LOOKAHEAD = 3
            for i in range(min(LOOKAHEAD, len(sched))):
                issue_S(i)
            for i in range(len(sched)):
                if i + LOOKAHEAD < len(sched):
                    issue_S(i + LOOKAHEAD)
                issue_A(i)
                jb, g = sched[i]
                if g[-1][0] == 4 * jb + 3:
                    issue_norm(jb)
    _split_sync_waits(nc)
    nc.finalize()
    return nc


def kernel(x, Wq, Wk, Wv, attention_mask=None, **_unused):
    from concourse.bass_utils import run_bass_kernel_spmd

    if "nc" not in _cache:
        _cache["nc"] = _build_nc()
    nc = _cache["nc"]

    wqvT = np.concatenate([np.asarray(Wq), np.asarray(Wv)], 0).T.astype(np.float16)
    wkT = np.asarray(Wk).T.astype(np.float16)
    # pre-shuffle weights so the device DMA is one contiguous transfer:
    # [E, cols] -> [128, EC*cols] with partition p holding rows {e*128+p}
    wqvh = np.ascontiguousarray(
        wqvT.reshape(EC, 128, 128).transpose(1, 0, 2).reshape(128, EC * 128))
    wkh = np.ascontiguousarray(
        wkT.reshape(EC, 128, D).transpose(1, 0, 2).reshape(128, EC * D))
    x = np.asarray(x)
    in_maps = [
        {
            "xT": np.ascontiguousarray(x[b].T).astype(np.float16),
            "wqvh": wqvh,
            "wkh": wkh,
        }
        for b in range(B)
    ]
    import os

    tmpdir = None
    if os.environ.get("BASS_TRACE"):
        tmpdir = os.environ.get("BASS_TRACE_DIR", "/tmp/bass_trace")
        os.makedirs(tmpdir, exist_ok=True)
    res = run_bass_kernel_spmd(nc, in_maps, core_ids=list(range(NCORES)), tmpdir=tmpdir)
    out = np.stack(
        [res.results[b]["out"].astype(np.float32) for b in range(B)], 0
    )
    _cache["last_exec_time_ns"] = res.exec_time_ns
    _cache["trace_dir"] = tmpdir
    return out
